# revision 8
# baseline (speedup 1.0000x reference)
"""Sliding-window entropy kernel for Trainium2 (Bass/Tile), 8-core data-parallel.

Math: for each length-64 window w of x along the last axis,
    out = sum_j p_j * log p_j ,  p = softmax(w)
        = U/S - log S ,  S = sum_j exp(w_j),  U = sum_j w_j * exp(w_j)
(no max-subtraction needed: inputs are ~N(0,1) so exp() is safe in fp32).

Per-core plan (each core gets 32 of the 256 (b,c) rows):
  1. DMA the 32x4096 shard into SBUF in a segment-interleaved layout
  2. PE transposes (8x 128x128) to a position-major layout
     Xp[pos_in_chunk=128, chunk*32 + row]
  3. ACT computes E=exp(Xp) PSUM->SBUF, DVE computes T=Xp*E
  4. PE band-matrix matmuls (float32r, full rate) produce the sliding sums
     S and U in PSUM: window starting at offset m inside a 128-chunk is a
     banded column of ones over partitions [m, m+63], split across the
     chunk boundary into two accumulated matmuls (bandA + bandB); the
     second reads the next chunk at free-offset +32.
  5. DVE/ACT: out = U * recip(S) - ln(S)
  6. PE transposes back to natural layout, DMA PSUM->DRAM (valid 4033 cols)
"""

import numpy as np

B, C, L = 32, 8, 4096
KWIN = 64
NCORES = 8
ROWS = B * C          # 256
RPC = ROWS // NCORES  # 32 rows per core
OUT_L = L - KWIN + 1  # 4033
F = 1024              # position-major free size per core (32 rows * 32 chunks)
PAD = 32              # zero tail so the last chunk's bandB reads zeros

TRACE = False
LAST = {}

_cache = {}


def _build_program():
    import concourse.bacc as bacc
    import concourse.bass as bass
    import concourse.tile as tile
    from concourse import mybir

    f32 = mybir.dt.float32
    f32r = mybir.dt.float32r
    EXP = mybir.ActivationFunctionType.Exp
    LN = mybir.ActivationFunctionType.Ln

    # Steer the act-table-load pass to the single combined exp+ln set:
    # the greedy picker takes the first set containing each func, which
    # would emit two table loads (~2.7us each on ACT). Editing the cached
    # set CONTENTS (never the order/indices, which walrus consumes) makes
    # it land on natural_log_exp_and_others for both.
    from concourse import hw_specs
    try:
        tabs = hw_specs.get_activation_tables("gen3")
        if "natural_log_exp_and_others" in tabs:
            combined = tabs["natural_log_exp_and_others"]
            assert EXP in combined and LN in combined
            for name, funcs in tabs.items():
                if name != "natural_log_exp_and_others":
                    funcs.discard(EXP)
                    funcs.discard(LN)
    except Exception:
        pass

    nc = bacc.Bacc("TRN2", target_bir_lowering=False)

    x = nc.dram_tensor("x", [RPC, L], f32r, kind="ExternalInput").ap()
    band_a = nc.dram_tensor("band_a", [128, 128], f32r, kind="ExternalInput").ap()
    band_b = nc.dram_tensor("band_b", [128, 128], f32r, kind="ExternalInput").ap()
    ident = nc.dram_tensor("ident", [128, 128], f32r, kind="ExternalInput").ap()
    zpad = nc.dram_tensor("zpad", [128, PAD], f32r, kind="ExternalInput").ap()
    out = nc.dram_tensor("out", [RPC, OUT_L], f32, kind="ExternalOutput").ap()

    with tile.TileContext(nc) as tc:
        with (
            tc.tile_pool(name="consts", bufs=1) as consts,
            tc.tile_pool(name="sb", bufs=1) as sb,
            tc.tile_pool(name="ps", bufs=1, space="PSUM") as ps,
        ):
            a_sb = consts.tile([128, 128], f32r)
            b_sb = consts.tile([128, 128], f32r)
            i_sb = consts.tile([128, 128], f32r)
            nc.sync.dma_start(out=a_sb[:], in_=band_a)
            nc.sync.dma_start(out=b_sb[:], in_=band_b)
            nc.sync.dma_start(out=i_sb[:], in_=ident)

            # Natural-side SBUF layout chosen so a plain 128-col PE transpose
            # lands position-major with chunk index c = b*4 + s:
            #   xn[q = s*32 + r, f = b*128 + g] = x[r, b*512 + s*128 + g]
            xn = sb.tile([128, F], f32r)
            for s in range(4):
                src = bass.AP(
                    tensor=x.tensor,
                    offset=x.offset + s * 128,
                    ap=[[L, RPC], [512, 8], [1, 128]],
                )
                nc.sync.dma_start(out=xn[s * 32:(s + 1) * 32, :], in_=src)

            xt = ps.tile([128, F], f32r)   # transposed input (position-major)
            s_ps = ps.tile([128, F], f32)  # window sums of E
            u_ps = ps.tile([128, F], f32)  # window sums of T
            onat = ps.tile([128, F], f32r) # output transposed back to natural

            ep = sb.tile([128, F + PAD], f32r)
            tp = sb.tile([128, F + PAD], f32r)
            nc.sync.dma_start(out=ep[:, F:F + PAD], in_=zpad)
            nc.sync.dma_start(out=tp[:, F:F + PAD], in_=zpad)

            for b8 in range(8):
                blk = slice(128 * b8, 128 * (b8 + 1))
                nc.tensor.transpose(
                    out=xt[:, blk],
                    in_=xn[:, blk],
                    identity=i_sb[:],
                )

            for h in range(2):
                sl = slice(512 * h, 512 * (h + 1))
                nc.scalar.activation(out=ep[:, sl], in_=xt[:, sl], func=EXP)
                nc.vector.tensor_mul(tp[:, sl], xt[:, sl], ep[:, sl])

            # Sliding sums: S[m, f'] = sum_p bandA[p,m]*E[p,f'] + bandB[p,m]*E[p,f'+32]
            # (f' = c*32 + r; next chunk of the same row is f'+32)
            for h in range(2):
                lo = 512 * h
                for src_t, dst in ((ep, s_ps), (tp, u_ps)):
                    nc.tensor.matmul(
                        dst[:, lo:lo + 512],
                        a_sb[:],
                        src_t[:, lo:lo + 512],
                        start=True, stop=False,
                    )
                    nc.tensor.matmul(
                        dst[:, lo:lo + 512],
                        b_sb[:],
                        src_t[:, lo + 32:lo + 544],
                        start=False, stop=True,
                    )

            rp = sb.tile([128, F], f32)
            lp = sb.tile([128, F], f32)
            pp = sb.tile([128, F], f32)
            op = sb.tile([128, F], f32r)
            nc.vector.reciprocal_approx_fast(out=rp[:], in_=s_ps[:])
            nc.scalar.activation(out=lp[:], in_=s_ps[:], func=LN)
            nc.vector.tensor_mul(pp[:], u_ps[:], rp[:])
            nc.vector.tensor_sub(op[:], pp[:], lp[:])

            for g in range(8):
                blk = slice(128 * g, 128 * (g + 1))
                nc.tensor.transpose(
                    out=onat[:, blk],
                    in_=op[:, blk],
                    identity=i_sb[:],
                )

            # DMA cannot read PSUM; bounce through SBUF on the scalar engine
            osb = sb.tile([128, F], f32)
            COPY = mybir.ActivationFunctionType.Copy
            for h in range(2):
                sl = slice(512 * h, 512 * (h + 1))
                nc.scalar.activation(out=osb[:, sl], in_=onat[:, sl], func=COPY)

            # osb[q'' = j*32 + r, g*128 + m] = out[r, (g*4+j)*128 + m]
            for j in range(3):
                dst = bass.AP(
                    tensor=out.tensor,
                    offset=out.offset + j * 128,
                    ap=[[OUT_L, RPC], [512, 8], [1, 128]],
                )
                nc.sync.dma_start(out=dst, in_=osb[j * 32:(j + 1) * 32, :])
            d2 = bass.AP(
                tensor=out.tensor,
                offset=out.offset + 384,
                ap=[[OUT_L, RPC], [512, 7], [1, 128]],
            )
            nc.sync.dma_start(out=d2, in_=osb[96:128, 0:896])
            d3 = bass.AP(
                tensor=out.tensor,
                offset=out.offset + 3968,
                ap=[[OUT_L, RPC], [1, 65]],
            )
            nc.sync.dma_start(out=d3, in_=osb[96:128, 896:961])

    nc.finalize()
    return nc


def _band_consts():
    p = np.arange(128)[:, None]
    m = np.arange(128)[None, :]
    a = ((p >= m) & (p < m + KWIN)).astype(np.float32)
    b = (p <= m - (KWIN + 1)).astype(np.float32)
    i = np.eye(128, dtype=np.float32)
    z = np.zeros((128, PAD), dtype=np.float32)
    return a, b, i, z


def get_program():
    if "nc" not in _cache:
        _cache["nc"] = _build_program()
    return _cache["nc"]


def kernel(k, input):
    assert int(k) == KWIN, f"kernel hardcoded for k={KWIN}, got {k}"
    from concourse.bass_utils import run_bass_kernel_spmd

    x = np.ascontiguousarray(np.asarray(input, dtype=np.float32).reshape(ROWS, L))
    nc = get_program()
    a, b, i, z = _band_consts()
    in_maps = [
        {
            "x": np.ascontiguousarray(x[c * RPC:(c + 1) * RPC]),
            "band_a": a,
            "band_b": b,
            "ident": i,
            "zpad": z,
        }
        for c in range(NCORES)
    ]
    res = run_bass_kernel_spmd(nc, in_maps, core_ids=list(range(NCORES)), trace=TRACE)
    LAST["results"] = res
    outs = np.stack([res.results[c]["out"] for c in range(NCORES)], axis=0)
    return np.ascontiguousarray(outs.reshape(B, C, OUT_L).astype(np.float32))


# revision 10
# speedup vs baseline: 1.0759x; 1.0759x over previous
"""Sliding-window entropy kernel for Trainium2 (Bass/Tile), 8-core data-parallel.

Math: for each length-64 window w of x along the last axis,
    out = sum_j p_j * log p_j ,  p = softmax(w)
        = U/S - log S ,  S = sum_j exp(w_j),  U = sum_j w_j * exp(w_j)
(no max-subtraction needed: inputs are ~N(0,1) so exp() is safe in fp32).

Per-core plan (each core gets 32 of the 256 (b,c) rows, 4 segments of 1024
positions per row laid across partitions q = s*32 + r):
  1. one fat contiguous DMA pair loads xn[q, f] = x[r, s*1024 + f]
  2. PE transposes (8x 128x128, f32r) land position-major:
     Xt[m, f' = b*128 + s*32 + r] = position chunk c = s*8 + b of row r
  3. ACT E=exp(Xt) PSUM->SBUF, DVE T=Xt*E
  4. PE band matmuls (f32r) compute sliding sums S,U into PSUM: window at
     offset m in a chunk = ones over partitions [m, m+63], split across the
     chunk boundary: bandA on the chunk + bandB on the next chunk, which
     lives at f'+128 (b<7) or f'-864 (b=7; garbage for s=3 is masked by
     zero band columns / discarded outputs)
  5. DVE/ACT per half: out = U * recip(S) - ln(S)
  6. PE transposes back; ACT copies PSUM->SBUF; two contiguous output DMAs
"""

import numpy as np

B, C, L = 32, 8, 4096
KWIN = 64
NCORES = 8
ROWS = B * C          # 256
RPC = ROWS // NCORES  # 32 rows per core
OUT_L = L - KWIN + 1  # 4033
F = 1024              # position-major free size per core (32 rows * 32 chunks)

TRACE = False
LAST = {}

_cache = {}


def _build_program():
    import concourse.bacc as bacc
    import concourse.bass as bass
    import concourse.tile as tile
    from concourse import mybir

    f32 = mybir.dt.float32
    f32r = mybir.dt.float32r
    EXP = mybir.ActivationFunctionType.Exp
    LN = mybir.ActivationFunctionType.Ln
    COPY = mybir.ActivationFunctionType.Copy

    # Steer the act-table-load pass to the single combined exp+ln set:
    # the greedy picker takes the first set containing each func, which
    # would emit two table loads (~1.5us each on ACT). Editing the cached
    # set CONTENTS (never the order/indices, which walrus consumes) makes
    # it land on natural_log_exp_and_others for both.
    from concourse import hw_specs
    try:
        tabs = hw_specs.get_activation_tables("gen3")
        if "natural_log_exp_and_others" in tabs:
            combined = tabs["natural_log_exp_and_others"]
            assert EXP in combined and LN in combined
            for name, funcs in tabs.items():
                if name != "natural_log_exp_and_others":
                    funcs.discard(EXP)
                    funcs.discard(LN)
    except Exception:
        pass

    nc = bacc.Bacc("TRN2", target_bir_lowering=False)

    x = nc.dram_tensor("x", [RPC, L], f32r, kind="ExternalInput").ap()
    # consts = [bandA | bandB | identity] side by side
    consts = nc.dram_tensor("consts", [128, 384], f32r, kind="ExternalInput").ap()
    out = nc.dram_tensor("out", [RPC, OUT_L], f32, kind="ExternalOutput").ap()

    with tile.TileContext(nc) as tc:
        with (
            tc.tile_pool(name="cp", bufs=1) as cp,
            tc.tile_pool(name="sb", bufs=1) as sb,
            tc.tile_pool(name="ps", bufs=1, space="PSUM") as ps,
        ):
            ct = cp.tile([128, 384], f32r)
            nc.sync.dma_start(out=ct[:], in_=consts)
            a_sb = ct[:, 0:128]
            b_sb = ct[:, 128:256]
            i_sb = ct[:, 256:384]

            # xn[q = s*32 + r, f] = x[r, s*1024 + f]; two halves for pipelining
            xn = sb.tile([128, F], f32r)
            for h in range(2):
                src = bass.AP(
                    tensor=x.tensor,
                    offset=x.offset + h * 512,
                    ap=[[1024, 4], [L, RPC], [1, 512]],
                )
                nc.sync.dma_start(out=xn[:, 512 * h:512 * (h + 1)], in_=src)

            xt = ps.tile([128, F], f32r)   # transposed input (position-major)
            s_ps = ps.tile([128, F], f32)  # window sums of E
            u_ps = ps.tile([128, F], f32)  # window sums of T
            onat = ps.tile([128, F], f32r)  # output transposed back to natural

            ep = sb.tile([128, F], f32r)
            tp = sb.tile([128, F], f32r)
            rp = sb.tile([128, F], f32)
            lp = sb.tile([128, F], f32)
            pp = sb.tile([128, F], f32)
            op = sb.tile([128, F], f32r)
            osb = sb.tile([128, F], f32)

            for b8 in range(8):
                blk = slice(128 * b8, 128 * (b8 + 1))
                nc.tensor.transpose(out=xt[:, blk], in_=xn[:, blk], identity=i_sb)

            for h in range(2):
                sl = slice(512 * h, 512 * (h + 1))
                nc.scalar.activation(out=ep[:, sl], in_=xt[:, sl], func=EXP)
                nc.vector.tensor_mul(tp[:, sl], xt[:, sl], ep[:, sl])

            # Sliding sums. bandB reads the next chunk: +128 for b<7;
            # the b=7 columns [896:1024) read [32:160) (= next segment's
            # first chunk; harmless garbage for s=3).
            def band_mms(arr, dst):
                # h0: columns [0:512) = b 0..3
                nc.tensor.matmul(dst[:, 0:512], a_sb, arr[:, 0:512],
                                 start=True, stop=False)
                nc.tensor.matmul(dst[:, 0:512], b_sb, arr[:, 128:640],
                                 start=False, stop=True)
                # h1: columns [512:1024) = b 4..7, one accumulation group
                nc.tensor.matmul(dst[:, 512:1024], a_sb, arr[:, 512:1024],
                                 start=True, stop=False)
                nc.tensor.matmul(dst[:, 512:896], b_sb, arr[:, 640:1024],
                                 start=False, stop=False)
                nc.tensor.matmul(dst[:, 896:1024], b_sb, arr[:, 32:160],
                                 start=False, stop=True)

            band_mms(ep, s_ps)
            band_mms(tp, u_ps)

            for h in range(2):
                sl = slice(512 * h, 512 * (h + 1))
                nc.vector.reciprocal_approx_fast(out=rp[:, sl], in_=s_ps[:, sl])
                nc.scalar.activation(out=lp[:, sl], in_=s_ps[:, sl], func=LN)
                nc.vector.tensor_mul(pp[:, sl], u_ps[:, sl], rp[:, sl])
                nc.vector.tensor_sub(op[:, sl], pp[:, sl], lp[:, sl])

            for g in range(8):
                blk = slice(128 * g, 128 * (g + 1))
                nc.tensor.transpose(out=onat[:, blk], in_=op[:, blk], identity=i_sb)

            # DMA cannot read PSUM; bounce through SBUF on the scalar engine
            for h in range(2):
                sl = slice(512 * h, 512 * (h + 1))
                nc.scalar.activation(out=osb[:, sl], in_=onat[:, sl], func=COPY)

            # osb[q = s*32 + r, g*128 + m] = out[r, s*1024 + g*128 + m]:
            # per-partition free dim is a contiguous 4KB run of one row.
            d1 = bass.AP(
                tensor=out.tensor,
                offset=out.offset,
                ap=[[1024, 3], [OUT_L, RPC], [1, F]],
            )
            nc.sync.dma_start(out=d1, in_=osb[0:96, :])
            # s=3 rows only have 961 valid outputs (t in [3072, 4033))
            d2 = bass.AP(
                tensor=out.tensor,
                offset=out.offset + 3072,
                ap=[[OUT_L, RPC], [1, 961]],
            )
            nc.sync.dma_start(out=d2, in_=osb[96:128, 0:961])

    nc.finalize()
    return nc


def _consts():
    p = np.arange(128)[:, None]
    m = np.arange(128)[None, :]
    a = ((p >= m) & (p < m + KWIN)).astype(np.float32)
    b = (p <= m - (KWIN + 1)).astype(np.float32)
    i = np.eye(128, dtype=np.float32)
    return np.ascontiguousarray(np.concatenate([a, b, i], axis=1))


def get_program():
    if "nc" not in _cache:
        _cache["nc"] = _build_program()
    return _cache["nc"]


def kernel(k, input):
    assert int(k) == KWIN, f"kernel hardcoded for k={KWIN}, got {k}"
    from concourse.bass_utils import run_bass_kernel_spmd

    x = np.ascontiguousarray(np.asarray(input, dtype=np.float32).reshape(ROWS, L))
    nc = get_program()
    ct = _consts()
    in_maps = [
        {"x": np.ascontiguousarray(x[c * RPC:(c + 1) * RPC]), "consts": ct}
        for c in range(NCORES)
    ]
    res = run_bass_kernel_spmd(nc, in_maps, core_ids=list(range(NCORES)), trace=TRACE)
    LAST["results"] = res
    outs = np.stack([res.results[c]["out"] for c in range(NCORES)], axis=0)
    return np.ascontiguousarray(outs.reshape(B, C, OUT_L).astype(np.float32))


# revision 12
# speedup vs baseline: 1.3228x; 1.2295x over previous
"""Sliding-window entropy kernel for Trainium2 (Bass/Tile), 8-core data-parallel.

Math: for each length-64 window w of x along the last axis,
    out = sum_j p_j * log p_j ,  p = softmax(w)
        = U/S - log S ,  S = sum_j exp(w_j),  U = sum_j w_j * exp(w_j)
(no max-subtraction needed: inputs are ~N(0,1) so exp() is safe in fp32).

Per-core plan (each core gets 32 of the 256 (b,c) rows, 4 segments of 1024
positions per row laid across partitions q = s*32 + r):
  1. one fat contiguous DMA pair loads xn[q, f] = x[r, s*1024 + f]
  2. PE transposes (8x 128x128, f32r) land position-major:
     Xt[m, f' = b*128 + s*32 + r] = position chunk c = s*8 + b of row r
  3. ACT E=exp(Xt) PSUM->SBUF, DVE T=Xt*E
  4. PE band matmuls (f32r) compute sliding sums S,U into PSUM: window at
     offset m in a chunk = ones over partitions [m, m+63], split across the
     chunk boundary: bandA on the chunk + bandB on the next chunk, which
     lives at f'+128 (b<7) or f'-864 (b=7; garbage for s=3 is masked by
     zero band columns / discarded outputs)
  5. DVE/ACT per half: out = U * recip(S) - ln(S)
  6. PE transposes back; ACT copies PSUM->SBUF; two contiguous output DMAs
"""

import numpy as np

B, C, L = 32, 8, 4096
KWIN = 64
NCORES = 8
ROWS = B * C          # 256
RPC = ROWS // NCORES  # 32 rows per core
OUT_L = L - KWIN + 1  # 4033
F = 1024              # position-major free size per core (32 rows * 32 chunks)

TRACE = False
LAST = {}

_cache = {}


def _build_program():
    import concourse.bacc as bacc
    import concourse.bass as bass
    import concourse.tile as tile
    from concourse import mybir

    f32 = mybir.dt.float32
    f32r = mybir.dt.float32r
    EXP = mybir.ActivationFunctionType.Exp
    LN = mybir.ActivationFunctionType.Ln
    COPY = mybir.ActivationFunctionType.Copy

    # Steer the act-table-load pass to the single combined exp+ln set:
    # the greedy picker takes the first set containing each func, which
    # would emit two table loads (~1.5us each on ACT). Editing the cached
    # set CONTENTS (never the order/indices, which walrus consumes) makes
    # it land on natural_log_exp_and_others for both.
    from concourse import hw_specs
    try:
        tabs = hw_specs.get_activation_tables("gen3")
        if "natural_log_exp_and_others" in tabs:
            combined = tabs["natural_log_exp_and_others"]
            assert EXP in combined and LN in combined
            for name, funcs in tabs.items():
                if name != "natural_log_exp_and_others":
                    funcs.discard(EXP)
                    funcs.discard(LN)
    except Exception:
        pass

    nc = bacc.Bacc("TRN2", target_bir_lowering=False)

    x = nc.dram_tensor("x", [RPC, L], f32r, kind="ExternalInput").ap()
    # consts = [bandA | bandB | identity] side by side
    consts = nc.dram_tensor("consts", [128, 384], f32r, kind="ExternalInput").ap()
    out = nc.dram_tensor("out", [RPC, OUT_L], f32, kind="ExternalOutput").ap()

    with tile.TileContext(nc) as tc:
        with (
            tc.tile_pool(name="cp", bufs=1) as cp,
            tc.tile_pool(name="sb", bufs=1) as sb,
            tc.tile_pool(name="ps", bufs=1, space="PSUM") as ps,
        ):
            ct = cp.tile([128, 384], f32r)
            nc.sync.dma_start(out=ct[:], in_=consts)
            a_sb = ct[:, 0:128]
            b_sb = ct[:, 128:256]
            i_sb = ct[:, 256:384]

            # xn[q = s*32 + r, f] = x[r, s*1024 + f]; split per (h, s) so each
            # DMA's outermost DRAM dim is 32 — the HWDGE fans work across the
            # 16 DMA engines by the outermost AP dim (4 would use 4 engines).
            xn = sb.tile([128, F], f32r)
            for h in range(2):
                for s in range(4):
                    src = bass.AP(
                        tensor=x.tensor,
                        offset=x.offset + s * 1024 + h * 512,
                        ap=[[L, RPC], [1, 512]],
                    )
                    nc.sync.dma_start(
                        out=xn[32 * s:32 * (s + 1), 512 * h:512 * (h + 1)], in_=src
                    )

            xt = ps.tile([128, F], f32r)   # transposed input (position-major)
            s_ps = ps.tile([128, F], f32)  # window sums of E
            u_ps = ps.tile([128, F], f32)  # window sums of T
            onat = ps.tile([128, F], f32r)  # output transposed back to natural

            ep = sb.tile([128, F], f32r)
            tp = sb.tile([128, F], f32r)
            rp = sb.tile([128, F], f32)
            lp = sb.tile([128, F], f32)
            pp = sb.tile([128, F], f32)
            op = sb.tile([128, F], f32r)
            osb = sb.tile([128, F], f32)

            for b8 in range(8):
                blk = slice(128 * b8, 128 * (b8 + 1))
                nc.tensor.transpose(out=xt[:, blk], in_=xn[:, blk], identity=i_sb)

            for h in range(2):
                sl = slice(512 * h, 512 * (h + 1))
                nc.scalar.activation(out=ep[:, sl], in_=xt[:, sl], func=EXP)
                nc.vector.tensor_mul(tp[:, sl], xt[:, sl], ep[:, sl])

            # Sliding sums. bandB reads the next chunk: +128 for b<7;
            # the b=7 columns [896:1024) read [32:160) (= next segment's
            # first chunk; harmless garbage for s=3).
            def band_mms(arr, dst):
                # h0: columns [0:512) = b 0..3
                nc.tensor.matmul(dst[:, 0:512], a_sb, arr[:, 0:512],
                                 start=True, stop=False)
                nc.tensor.matmul(dst[:, 0:512], b_sb, arr[:, 128:640],
                                 start=False, stop=True)
                # h1: columns [512:1024) = b 4..7, one accumulation group
                nc.tensor.matmul(dst[:, 512:1024], a_sb, arr[:, 512:1024],
                                 start=True, stop=False)
                nc.tensor.matmul(dst[:, 512:896], b_sb, arr[:, 640:1024],
                                 start=False, stop=False)
                nc.tensor.matmul(dst[:, 896:1024], b_sb, arr[:, 32:160],
                                 start=False, stop=True)

            band_mms(ep, s_ps)
            band_mms(tp, u_ps)

            for h in range(2):
                sl = slice(512 * h, 512 * (h + 1))
                nc.vector.reciprocal_approx_fast(out=rp[:, sl], in_=s_ps[:, sl])
                nc.scalar.activation(out=lp[:, sl], in_=s_ps[:, sl], func=LN)
                nc.vector.tensor_mul(pp[:, sl], u_ps[:, sl], rp[:, sl])
                nc.vector.tensor_sub(op[:, sl], pp[:, sl], lp[:, sl])

            for g in range(8):
                blk = slice(128 * g, 128 * (g + 1))
                nc.tensor.transpose(out=onat[:, blk], in_=op[:, blk], identity=i_sb)

            # DMA cannot read PSUM; bounce through SBUF on the scalar engine
            for h in range(2):
                sl = slice(512 * h, 512 * (h + 1))
                nc.scalar.activation(out=osb[:, sl], in_=onat[:, sl], func=COPY)

            # osb[q = s*32 + r, g*128 + m] = out[r, s*1024 + g*128 + m]:
            # per-partition free dim is a contiguous 4KB run of one row.
            # One DMA per s keeps the outer DRAM dim at 32 (full engine fanout).
            for s in range(3):
                dst = bass.AP(
                    tensor=out.tensor,
                    offset=out.offset + s * 1024,
                    ap=[[OUT_L, RPC], [1, F]],
                )
                nc.sync.dma_start(out=dst, in_=osb[32 * s:32 * (s + 1), :])
            # s=3 rows only have 961 valid outputs (t in [3072, 4033))
            d2 = bass.AP(
                tensor=out.tensor,
                offset=out.offset + 3072,
                ap=[[OUT_L, RPC], [1, 961]],
            )
            nc.sync.dma_start(out=d2, in_=osb[96:128, 0:961])

    nc.finalize()
    return nc


def _consts():
    p = np.arange(128)[:, None]
    m = np.arange(128)[None, :]
    a = ((p >= m) & (p < m + KWIN)).astype(np.float32)
    b = (p <= m - (KWIN + 1)).astype(np.float32)
    i = np.eye(128, dtype=np.float32)
    return np.ascontiguousarray(np.concatenate([a, b, i], axis=1))


def get_program():
    if "nc" not in _cache:
        _cache["nc"] = _build_program()
    return _cache["nc"]


def kernel(k, input):
    assert int(k) == KWIN, f"kernel hardcoded for k={KWIN}, got {k}"
    from concourse.bass_utils import run_bass_kernel_spmd

    x = np.ascontiguousarray(np.asarray(input, dtype=np.float32).reshape(ROWS, L))
    nc = get_program()
    ct = _consts()
    in_maps = [
        {"x": np.ascontiguousarray(x[c * RPC:(c + 1) * RPC]), "consts": ct}
        for c in range(NCORES)
    ]
    res = run_bass_kernel_spmd(nc, in_maps, core_ids=list(range(NCORES)), trace=TRACE)
    LAST["results"] = res
    outs = np.stack([res.results[c]["out"] for c in range(NCORES)], axis=0)
    return np.ascontiguousarray(outs.reshape(B, C, OUT_L).astype(np.float32))


# revision 17
# speedup vs baseline: 1.4691x; 1.1107x over previous
"""Sliding-window entropy kernel for Trainium2 (Bass/Tile), 8-core data-parallel.

Math: for each length-64 window w of x along the last axis,
    out = sum_j p_j * log p_j ,  p = softmax(w)
        = U/S - log S ,  S = sum_j exp(w_j),  U = sum_j w_j * exp(w_j)
(no max-subtraction needed: inputs are ~N(0,1) so exp() is safe in fp32).

Per-core plan (each core gets 32 of the 256 (b,c) rows; row r is cut into 4
segments of 1024 positions living on partitions q = r*4 + s):
  1. two fat DMAs load xn[q, f] = x[r, s*1024 + f] (outer DRAM dim 32 ->
     full 16-DMA-engine fanout)
  2. PE transposes (8x 128x128, f32r) land position-major:
     Xt[m, f' = b*128 + s*32 + r] = position chunk c = s*8 + b of row r\n     (the transpose's moving operand is a permutation matrix)
  3. ACT E=exp(Xt)->bf16, DVE T=Xt*E->bf16
  4. PE band matmuls (bf16, fp32 accum) compute sliding sums S,U in PSUM:
     window at offset m in a chunk = ones over partitions [m, m+63], split
     at the chunk boundary: bandA on the chunk + bandB on the next chunk,
     which lives at f'+128 (b<7) or f'-864 (b=7; the s=3 garbage is masked
     by zero band columns / discarded outputs)
  5. DVE/ACT per half: out = U * recip(S) - ln(S)
  6. PE transposes back; ACT+DVE copy PSUM->SBUF; 4 contiguous output DMAs
     (strided-partition source, 2 per HWDGE ring)
"""

import numpy as np

B, C, L = 32, 8, 4096
KWIN = 64
NCORES = 8
ROWS = B * C          # 256
RPC = ROWS // NCORES  # 32 rows per core
OUT_L = L - KWIN + 1  # 4033
F = 1024              # position-major free size per core (32 rows * 32 chunks)

TRACE = False
LAST = {}

_cache = {}


def _build_program():
    import concourse.bacc as bacc
    import concourse.bass as bass
    import concourse.tile as tile
    from concourse import mybir

    f32 = mybir.dt.float32
    f32r = mybir.dt.float32r
    bf16 = mybir.dt.bfloat16
    EXP = mybir.ActivationFunctionType.Exp
    LN = mybir.ActivationFunctionType.Ln
    COPY = mybir.ActivationFunctionType.Copy

    # Steer the act-table-load pass to the single combined exp+ln set:
    # the greedy picker takes the first set containing each func, which
    # would emit two table loads (~1.5us each on ACT). Editing the cached
    # set CONTENTS (never the order/indices, which walrus consumes) makes
    # it land on natural_log_exp_and_others for both.
    from concourse import hw_specs
    try:
        tabs = hw_specs.get_activation_tables("gen3")
        if "natural_log_exp_and_others" in tabs:
            combined = tabs["natural_log_exp_and_others"]
            assert EXP in combined and LN in combined
            for name, funcs in tabs.items():
                if name != "natural_log_exp_and_others":
                    funcs.discard(EXP)
                    funcs.discard(LN)
    except Exception:
        pass

    nc = bacc.Bacc("TRN2", target_bir_lowering=False)

    x = nc.dram_tensor("x", [RPC, L], f32r, kind="ExternalInput").ap()
    permid = nc.dram_tensor("permid", [128, 256], f32r, kind="ExternalInput").ap()
    bands = nc.dram_tensor("bands", [128, 256], bf16, kind="ExternalInput").ap()
    out = nc.dram_tensor("out", [RPC, OUT_L], f32, kind="ExternalOutput").ap()

    with tile.TileContext(nc) as tc:
        with (
            tc.tile_pool(name="cp", bufs=1) as cp,
            tc.tile_pool(name="sb", bufs=1) as sb,
            tc.tile_pool(name="ps", bufs=1, space="PSUM") as ps,
        ):
            pi_sb = cp.tile([128, 256], f32r)
            ab_sb = cp.tile([128, 256], bf16)
            nc.sync.dma_start(out=pi_sb[:], in_=permid)
            nc.sync.dma_start(out=ab_sb[:], in_=bands)
            p_sb = pi_sb[:, 0:128]
            i_sb = pi_sb[:, 128:256]
            a_sb = ab_sb[:, 0:128]
            b_sb = ab_sb[:, 128:256]

            # xn[q = r*4 + s, f] = x[r, s*1024 + f]; two column halves so the
            # first transposes can start while the second half streams in.
            xn = sb.tile([128, F], f32r)
            for h in range(2):
                src = bass.AP(
                    tensor=x.tensor,
                    offset=x.offset + h * 512,
                    ap=[[L, RPC], [1024, 4], [1, 512]],
                )
                nc.sync.dma_start(out=xn[:, 512 * h:512 * (h + 1)], in_=src)

            xt = ps.tile([128, F], f32r)   # transposed input (position-major)
            s_ps = ps.tile([128, F], f32)  # window sums of E
            u_ps = ps.tile([128, F], f32)  # window sums of T
            onat = ps.tile([128, F], f32r)  # output transposed back to natural

            ep = sb.tile([128, F], bf16)
            tp = sb.tile([128, F], bf16)
            rp = sb.tile([128, F], f32)
            lp = sb.tile([128, F], f32)
            pp = sb.tile([128, F], f32)
            op = sb.tile([128, F], f32r)
            osb = sb.tile([128, F], f32)

            # moving operand P permutes the output free order so that
            # f' = b*128 + s*32 + r even though xn partitions are r-major
            for b8 in range(8):
                blk = slice(128 * b8, 128 * (b8 + 1))
                nc.tensor.transpose(out=xt[:, blk], in_=xn[:, blk], identity=p_sb)

            for h in range(2):
                sl = slice(512 * h, 512 * (h + 1))
                nc.scalar.activation(out=ep[:, sl], in_=xt[:, sl], func=EXP)
                nc.vector.tensor_mul(tp[:, sl], xt[:, sl], ep[:, sl])

            # Sliding sums. bandB reads the next chunk: +128 for b<7; the
            # b=7 columns [896:1024) read the next segment's first chunk at
            # f'-895 (harmless finite garbage for s=3).
            def band_mms(arr, dst):
                nc.tensor.matmul(dst[:, 0:512], a_sb, arr[:, 0:512],
                                 start=True, stop=False)
                nc.tensor.matmul(dst[:, 0:512], b_sb, arr[:, 128:640],
                                 start=False, stop=True)
                nc.tensor.matmul(dst[:, 512:1024], a_sb, arr[:, 512:1024],
                                 start=True, stop=False)
                nc.tensor.matmul(dst[:, 512:896], b_sb, arr[:, 640:1024],
                                 start=False, stop=False)
                nc.tensor.matmul(dst[:, 896:1024], b_sb, arr[:, 32:160],
                                 start=False, stop=True)

            band_mms(ep, s_ps)
            band_mms(tp, u_ps)

            for h in range(2):
                sl = slice(512 * h, 512 * (h + 1))
                nc.vector.reciprocal_approx_fast(out=rp[:, sl], in_=s_ps[:, sl])
                nc.scalar.activation(out=lp[:, sl], in_=s_ps[:, sl], func=LN)
                nc.vector.tensor_mul(pp[:, sl], u_ps[:, sl], rp[:, sl])
                nc.vector.tensor_sub(op[:, sl], pp[:, sl], lp[:, sl])

            for g in range(8):
                blk = slice(128 * g, 128 * (g + 1))
                nc.tensor.transpose(out=onat[:, blk], in_=op[:, blk], identity=i_sb)

            # DMA cannot read PSUM; bounce through SBUF (ACT h0 / DVE h1)
            cp0 = nc.scalar.activation(out=osb[:, 0:512], in_=onat[:, 0:512],
                                       func=COPY)
            cp1 = nc.vector.tensor_copy(osb[:, 512:1024], onat[:, 512:1024])

            # osb[q = s*32 + r, g*128 + m] = out[r, s*1024 + g*128 + m]:
            # per-partition free dim is a contiguous 4KB run of one row.
            # One DMA per s (strided-partition source), spread over both
            # HWDGE rings. s=3 rows only have 961 valid outputs.
            for s in range(4):
                n = F if s < 3 else 961
                dst = bass.AP(
                    tensor=out.tensor,
                    offset=out.offset + s * 1024,
                    ap=[[OUT_L, RPC], [1, n]],
                )
                eng = nc.sync if s % 2 == 0 else nc.scalar
                eng.dma_start(out=dst, in_=osb[32 * s:32 * (s + 1), 0:n])

    nc.finalize()
    return nc


def _consts():
    p = np.arange(128)[:, None]
    m = np.arange(128)[None, :]
    a = ((p >= m) & (p < m + KWIN)).astype(np.float32)
    b = (p <= m - (KWIN + 1)).astype(np.float32)
    import ml_dtypes
    bands = np.ascontiguousarray(
        np.concatenate([a, b], axis=1).astype(ml_dtypes.bfloat16))
    q = np.arange(128)
    perm = np.zeros((128, 128), np.float32)
    perm[q, (q % 4) * 32 + q // 4] = 1.0
    ident = np.eye(128, dtype=np.float32)
    permid = np.ascontiguousarray(np.concatenate([perm, ident], axis=1))
    return permid, bands


def get_program():
    if "nc" not in _cache:
        _cache["nc"] = _build_program()
    return _cache["nc"]


def kernel(k, input):
    assert int(k) == KWIN, f"kernel hardcoded for k={KWIN}, got {k}"
    from concourse.bass_utils import run_bass_kernel_spmd

    x = np.ascontiguousarray(np.asarray(input, dtype=np.float32).reshape(ROWS, L))
    nc = get_program()
    permid, bands = _consts()
    in_maps = [
        {"x": np.ascontiguousarray(x[c * RPC:(c + 1) * RPC]),
         "permid": permid, "bands": bands}
        for c in range(NCORES)
    ]
    res = run_bass_kernel_spmd(nc, in_maps, core_ids=list(range(NCORES)), trace=TRACE)
    LAST["results"] = res
    outs = np.stack([res.results[c]["out"] for c in range(NCORES)], axis=0)
    return np.ascontiguousarray(outs.reshape(B, C, OUT_L).astype(np.float32))


# revision 18
# speedup vs baseline: 1.4916x; 1.0153x over previous
"""Sliding-window entropy kernel for Trainium2 (Bass/Tile), 8-core data-parallel.

Math: for each length-64 window w of x along the last axis,
    out = sum_j p_j * log p_j ,  p = softmax(w)
        = U/S - log S ,  S = sum_j exp(w_j),  U = sum_j w_j * exp(w_j)
(no max-subtraction needed: inputs are ~N(0,1) so exp() is safe in fp32).

Per-core plan (each core gets 32 of the 256 (b,c) rows; row r is cut into 4
segments of 1024 positions living on partitions q = r*4 + s):
  1. two fat DMAs load xn[q, f] = x[r, s*1024 + f] (outer DRAM dim 32 ->
     full 16-DMA-engine fanout)
  2. PE transposes (8x 128x128, f32r) land position-major:
     Xt[m, f' = b*128 + s*32 + r] = position chunk c = s*8 + b of row r\n     (the transpose's moving operand is a permutation matrix)
  3. ACT E=exp(Xt)->bf16, DVE T=Xt*E->bf16
  4. PE band matmuls (bf16, fp32 accum) compute sliding sums S,U in PSUM:
     window at offset m in a chunk = ones over partitions [m, m+63], split
     at the chunk boundary: bandA on the chunk + bandB on the next chunk,
     which lives at f'+128 (b<7) or f'-864 (b=7; the s=3 garbage is masked
     by zero band columns / discarded outputs)
  5. DVE/ACT per half: out = U * recip(S) - ln(S)
  6. PE transposes back; ACT+DVE copy PSUM->SBUF; 4 contiguous output DMAs
     (strided-partition source, 2 per HWDGE ring)
"""

import numpy as np

B, C, L = 32, 8, 4096
KWIN = 64
NCORES = 8
ROWS = B * C          # 256
RPC = ROWS // NCORES  # 32 rows per core
OUT_L = L - KWIN + 1  # 4033
F = 1024              # position-major free size per core (32 rows * 32 chunks)

TRACE = False
LAST = {}

_cache = {}


def _build_program():
    import concourse.bacc as bacc
    import concourse.bass as bass
    import concourse.tile as tile
    from concourse import mybir

    f32 = mybir.dt.float32
    f32r = mybir.dt.float32r
    bf16 = mybir.dt.bfloat16
    EXP = mybir.ActivationFunctionType.Exp
    LN = mybir.ActivationFunctionType.Ln
    COPY = mybir.ActivationFunctionType.Copy

    # Steer the act-table-load pass to the single combined exp+ln set:
    # the greedy picker takes the first set containing each func, which
    # would emit two table loads (~1.5us each on ACT). Editing the cached
    # set CONTENTS (never the order/indices, which walrus consumes) makes
    # it land on natural_log_exp_and_others for both.
    from concourse import hw_specs
    try:
        tabs = hw_specs.get_activation_tables("gen3")
        if "natural_log_exp_and_others" in tabs:
            combined = tabs["natural_log_exp_and_others"]
            assert EXP in combined and LN in combined
            for name, funcs in tabs.items():
                if name != "natural_log_exp_and_others":
                    funcs.discard(EXP)
                    funcs.discard(LN)
    except Exception:
        pass

    nc = bacc.Bacc("TRN2", target_bir_lowering=False)

    x = nc.dram_tensor("x", [RPC, L], f32r, kind="ExternalInput").ap()
    permid = nc.dram_tensor("permid", [128, 256], f32r, kind="ExternalInput").ap()
    bands = nc.dram_tensor("bands", [128, 256], bf16, kind="ExternalInput").ap()
    out = nc.dram_tensor("out", [RPC, OUT_L], f32, kind="ExternalOutput").ap()

    with tile.TileContext(nc) as tc:
        with (
            tc.tile_pool(name="cp", bufs=1) as cp,
            tc.tile_pool(name="sb", bufs=1) as sb,
            tc.tile_pool(name="ps", bufs=1, space="PSUM") as ps,
        ):
            pi_sb = cp.tile([128, 256], f32r)
            ab_sb = cp.tile([128, 256], bf16)
            nc.gpsimd.dma_start(out=pi_sb[:], in_=permid)
            nc.gpsimd.dma_start(out=ab_sb[:], in_=bands)
            p_sb = pi_sb[:, 0:128]
            i_sb = pi_sb[:, 128:256]
            a_sb = ab_sb[:, 0:128]
            b_sb = ab_sb[:, 128:256]

            # xn[q = r*4 + s, f] = x[r, s*1024 + f]; two column halves so the
            # first transposes can start while the second half streams in.
            xn = sb.tile([128, F], f32r)
            for lo, hi in ((0, 640), (640, 1024)):
                src = bass.AP(
                    tensor=x.tensor,
                    offset=x.offset + lo,
                    ap=[[L, RPC], [1024, 4], [1, hi - lo]],
                )
                nc.sync.dma_start(out=xn[:, lo:hi], in_=src)

            xt = ps.tile([128, F], f32r)   # transposed input (position-major)
            s_ps = ps.tile([128, F], f32)  # window sums of E
            u_ps = ps.tile([128, F], f32)  # window sums of T
            onat = ps.tile([128, F], f32r)  # output transposed back to natural

            ep = sb.tile([128, F], bf16)
            tp = sb.tile([128, F], bf16)
            rp = sb.tile([128, F], f32)
            lp = sb.tile([128, F], f32)
            pp = sb.tile([128, F], f32)
            op = sb.tile([128, F], f32r)
            osb = sb.tile([128, F], f32)

            # moving operand P permutes the output free order so that
            # f' = b*128 + s*32 + r even though xn partitions are r-major
            for b8 in range(8):
                blk = slice(128 * b8, 128 * (b8 + 1))
                nc.tensor.transpose(out=xt[:, blk], in_=xn[:, blk], identity=p_sb)

            for lo, hi in ((0, 640), (640, 1024)):
                sl = slice(lo, hi)
                nc.scalar.activation(out=ep[:, sl], in_=xt[:, sl], func=EXP)
                nc.vector.tensor_mul(tp[:, sl], xt[:, sl], ep[:, sl])

            # Sliding sums. bandB reads the next chunk: +128 for b<7; the
            # b=7 columns [896:1024) read the next segment's first chunk at
            # f'-895 (harmless finite garbage for s=3).
            def band_mms(arr, dst, h):
                if h == 0:
                    nc.tensor.matmul(dst[:, 0:512], a_sb, arr[:, 0:512],
                                     start=True, stop=False)
                    nc.tensor.matmul(dst[:, 0:512], b_sb, arr[:, 128:640],
                                     start=False, stop=True)
                else:
                    nc.tensor.matmul(dst[:, 512:1024], a_sb, arr[:, 512:1024],
                                     start=True, stop=False)
                    nc.tensor.matmul(dst[:, 512:896], b_sb, arr[:, 640:1024],
                                     start=False, stop=False)
                    nc.tensor.matmul(dst[:, 896:1024], b_sb, arr[:, 32:160],
                                     start=False, stop=True)

            band_mms(ep, s_ps, 0)
            band_mms(tp, u_ps, 0)
            band_mms(ep, s_ps, 1)
            band_mms(tp, u_ps, 1)

            for h in range(2):
                sl = slice(512 * h, 512 * (h + 1))
                nc.vector.reciprocal_approx_fast(out=rp[:, sl], in_=s_ps[:, sl])
                nc.scalar.activation(out=lp[:, sl], in_=s_ps[:, sl], func=LN)
                nc.vector.tensor_mul(pp[:, sl], u_ps[:, sl], rp[:, sl])
                nc.vector.tensor_sub(op[:, sl], pp[:, sl], lp[:, sl])

            for g in range(8):
                blk = slice(128 * g, 128 * (g + 1))
                nc.tensor.transpose(out=onat[:, blk], in_=op[:, blk], identity=i_sb)

            # DMA cannot read PSUM; bounce through SBUF (ACT h0 / DVE h1)
            cp0 = nc.scalar.activation(out=osb[:, 0:512], in_=onat[:, 0:512],
                                       func=COPY)
            cp1 = nc.vector.tensor_copy(osb[:, 512:1024], onat[:, 512:1024])

            # osb[q = s*32 + r, g*128 + m] = out[r, s*1024 + g*128 + m]:
            # per-partition free dim is a contiguous 4KB run of one row.
            # One DMA per s (strided-partition source), spread over both
            # HWDGE rings. s=3 rows only have 961 valid outputs.
            for s in range(4):
                n = F if s < 3 else 961
                dst = bass.AP(
                    tensor=out.tensor,
                    offset=out.offset + s * 1024,
                    ap=[[OUT_L, RPC], [1, n]],
                )
                nc.sync.dma_start(out=dst, in_=osb[32 * s:32 * (s + 1), 0:n])

    nc.finalize()
    return nc


def _consts():
    p = np.arange(128)[:, None]
    m = np.arange(128)[None, :]
    a = ((p >= m) & (p < m + KWIN)).astype(np.float32)
    b = (p <= m - (KWIN + 1)).astype(np.float32)
    import ml_dtypes
    bands = np.ascontiguousarray(
        np.concatenate([a, b], axis=1).astype(ml_dtypes.bfloat16))
    q = np.arange(128)
    perm = np.zeros((128, 128), np.float32)
    perm[q, (q % 4) * 32 + q // 4] = 1.0
    ident = np.eye(128, dtype=np.float32)
    permid = np.ascontiguousarray(np.concatenate([perm, ident], axis=1))
    return permid, bands


def get_program():
    if "nc" not in _cache:
        _cache["nc"] = _build_program()
    return _cache["nc"]


def kernel(k, input):
    assert int(k) == KWIN, f"kernel hardcoded for k={KWIN}, got {k}"
    from concourse.bass_utils import run_bass_kernel_spmd

    x = np.ascontiguousarray(np.asarray(input, dtype=np.float32).reshape(ROWS, L))
    nc = get_program()
    permid, bands = _consts()
    in_maps = [
        {"x": np.ascontiguousarray(x[c * RPC:(c + 1) * RPC]),
         "permid": permid, "bands": bands}
        for c in range(NCORES)
    ]
    res = run_bass_kernel_spmd(nc, in_maps, core_ids=list(range(NCORES)), trace=TRACE)
    LAST["results"] = res
    outs = np.stack([res.results[c]["out"] for c in range(NCORES)], axis=0)
    return np.ascontiguousarray(outs.reshape(B, C, OUT_L).astype(np.float32))
